# revision 4
# baseline (speedup 1.0000x reference)
"""MultiHeadDistanceKNN Trainium2 kernel, v2.

kernel(x, W) -> adj : x [2,2048,512] f32, W [4,512,128] f32 -> adj [2,2048,2048] f32.

8 cores = 4 heads x 2 batches; core i handles (h=i//2, b=i%2) and computes
  C[n,m] = exp(-d2[n,m]/(2*mu^2)) * 1[d2[n,m] <= max(T_n, T_m)]
for the UPPER-TRIANGLE row slabs only (C is symmetric); host mirrors and
takes the head-mean. Output DMA'd as bf16.

d2 built on PE with ONE fp32r matmul per [128,512] tile (full-rate for
free dim >= 512): psum = (-2z)^T z; a DVE stt adds sq_n (per-partition
scalar) + sq_m (broadcast row) while copying psum -> SBUF. z itself is
computed with fp32 matmuls (fp32r there flips ~0.5e-2 of near-threshold
kNN edges).

T_n (K-th smallest of row n, K=307): exact per-row mean mu_n and model
std sig_n -> gaussian-quantile t0; count at t0 fused into the build
(DVE is_le+accum / ACT sign+accum split per chunk, per-engine scratch
pools); then 4 model-slope secant rounds with staged aims (K+25, K+6,
K+2.5, K+2.5), tracking the tightest t with count >= K; exact
top-8-below-hi finisher: s = sign(hi - d2) on ACT, w = d2*s on
DVE/gpsimd, DVE max8 -> T = value at exact rank K (w8[7] / hi
fallbacks otherwise). exp slabs (bf16) are precomputed in ACT slack
during rounds/finisher; only upper-triangle slabs are masked,
multiplied and DMA'd (bf16); host mirrors and takes the head-mean.
"""
import numpy as np

import concourse.bass as bass
import concourse.mybir as mybir
from concourse import bacc
from concourse.tile import TileContext
from concourse.masks import make_identity

F32 = mybir.dt.float32
F32R = mybir.dt.float32r
F16 = mybir.dt.float16
BF16 = mybir.dt.bfloat16
U8 = mybir.dt.uint8
Alu = mybir.AluOpType
Act = mybir.ActivationFunctionType
X_AX = mybir.AxisListType.X
XY_AX = mybir.AxisListType.XY

N = 2048
D = 512
NCH = 16
NJT = 4
K = 307

ZEFF = -1.0316          # gaussian z for quantile (K+2.5)/N
PHI = 0.2337            # normal pdf at ZEFF
AIMS = [25.0, 6.0, 2.5, 2.5]   # secant-round count aims (above K)
STEP_CLAMP = 0.6        # max secant step in units of sigma_n
BIGW = 4096.0           # rank-mask offset; d2 << 4096 so order is preserved
BIGT = float(2.0 ** 60)


def AUG_CHUNK(c):
    return False          # PE-aug chunks; others add sq via DVE stt


def CNT_ACT_BUILD(c):
    return c >= 2         # r0 count on ACT for these chunks


def CNT_ACT_ROUND(c):
    return c >= 8         # round counts on ACT for these chunks


def FIN_GPS(c):
    return c % 2 == 1     # finisher w-build on gpsimd for these chunks


def MUL_GPS(c):
    return c >= 6         # final multiply on gpsimd (narrow slabs)


def build_nc():
    nc = bacc.Bacc("TRN2", target_bir_lowering=False)
    xb = nc.dram_tensor("xb", [N, D], F32, kind="ExternalInput")
    wh = nc.dram_tensor("wh", [D, 128], F32, kind="ExternalInput")
    outp = nc.dram_tensor("outp", [N, N], BF16, kind="ExternalOutput")

    with TileContext(nc) as tc:
        with tc.tile_pool(name="base", bufs=1) as base, \
             tc.tile_pool(name="st", bufs=1) as st:
            D2 = base.tile([128, NCH * N], F32)
            ident = base.tile([128, 128], F32)
            make_identity(nc, ident[:])
            ones_col = base.tile([128, 1], F32)
            nc.gpsimd.memset(ones_col[:], 1.0)
            ones_row = base.tile([1, 128], F32)
            nc.gpsimd.memset(ones_row[:], 1.0)
            id1 = base.tile([1, 1], F32)
            nc.gpsimd.memset(id1[:], 1.0)
            c005 = base.tile([128, 1], F32)
            nc.gpsimd.memset(c005[:], 0.05)

            def stt16(name, w=NCH, dt=F32):
                return st.tile([128, w], dt, tag=name, name=name)
            sqcol = stt16("sqcol"); zdots = stt16("zdots")
            mu = stt16("mu"); sig = stt16("sig"); slp = stt16("slp")
            tcur = stt16("tcur"); ccur = stt16("ccur"); tneg = stt16("tneg")
            hiv = stt16("hiv"); chiv = stt16("chiv")
            sacc = stt16("sacc"); musum = stt16("musum")
            Tfin = stt16("Tfin")
            tmp1 = stt16("tmp1"); tmp2 = stt16("tmp2"); tmp3 = stt16("tmp3")
            tmp4 = stt16("tmp4")
            mge = stt16("mge", NCH, U8)
            mbh = stt16("mbh", NCH, U8)
            mok = stt16("mok", NCH, U8)
            mnew = stt16("mnew", NCH, U8)
            s_vec = st.tile([128, 1], F32, tag="s_vec", name="s_vec")
            s1b = st.tile([128, 1], F32, tag="s1b", name="s1b")
            neginvb = st.tile([128, 1], F32, tag="neginvb", name="neginvb")
            vglob = st.tile([128, 1], F32, tag="vglob", name="vglob")
            sc1 = st.tile([1, 1], F32, tag="sc1", name="sc1")
            sc2 = st.tile([1, 1], F32, tag="sc2", name="sc2")
            sc3 = st.tile([1, 1], F32, tag="sc3", name="sc3")
            sc4 = st.tile([1, 1], F32, tag="sc4", name="sc4")
            scv = st.tile([1, 1], F32, tag="scv", name="scv")
            w8 = st.tile([128, NCH * 8], F32, tag="w8", name="w8")
            iota8f = st.tile([128, NCH * 8], F32, tag="iota8f", name="iota8f")
            nc.gpsimd.iota(iota8f[:], pattern=[[0, NCH], [1, 8]], base=0,
                           channel_multiplier=0,
                           allow_small_or_imprecise_dtypes=True)
            ohsel = st.tile([128, NCH * 8], F32, tag="ohsel", name="ohsel")

            # ================= prep + build (mid scope) =================
            with tc.tile_pool(name="mid", bufs=1) as mid:
                sqrow = mid.tile([1, N], F32, tag="sqrow", name="sqrow")
                SQROWB = mid.tile([128, N], F32, tag="SQROWB", name="SQROWB")
                zrL = mid.tile([128, N], F32R, tag="zrL", name="zrL")  # -2z
                zrR = mid.tile([128, N], F32R, tag="zrR", name="zrR")  # z

                with tc.tile_pool(name="prep", bufs=8) as prep, \
                     tc.tile_pool(name="prep1", bufs=1) as prep1, \
                     tc.tile_pool(name="pps", bufs=4, space="PSUM") as pps, \
                     tc.tile_pool(name="pps1", bufs=4, space="PSUM") as pps1:
                    w_sb = prep1.tile([128, D], F32)
                    for dc in range(4):
                        nc.gpsimd.dma_start(w_sb[:, dc * 128:(dc + 1) * 128],
                                            wh[dc * 128:(dc + 1) * 128, :])
                    # two-half z build: xt tiles hold 2 dc-slices at a time
                    xt = [prep1.tile([128, N], F32, tag=f"xt{i}",
                                     name=f"xt{i}") for i in range(2)]
                    zt_ps = [pps.tile([128, 512], F32, tag="big",
                                      name=f"zt{j}") for j in range(NJT)]
                    for half in range(2):
                        for c in range(NCH):
                            x_sb = prep.tile([128, D // 2], F32, tag="x_sb",
                                             name="x_sb")
                            dmae = [nc.gpsimd, nc.sync,
                                    nc.scalar][(half * NCH + c) % 3]
                            dmae.dma_start(
                                x_sb[:],
                                xb[c * 128:(c + 1) * 128,
                                   half * 256:(half + 1) * 256])
                            for i in range(2):
                                tr_ps = pps1.tile([128, 128], F32, tag="small",
                                                  name="tr")
                                nc.tensor.transpose(
                                    tr_ps[:], x_sb[:, i * 128:(i + 1) * 128],
                                    ident[:])
                                if i % 2 == 0:
                                    nc.scalar.copy(
                                        xt[i][:, c * 128:(c + 1) * 128],
                                        tr_ps[:])
                                else:
                                    nc.vector.tensor_copy(
                                        xt[i][:, c * 128:(c + 1) * 128],
                                        tr_ps[:])
                            if c % 4 == 3:
                                # cols for j-slice c//4 are ready
                                j = c // 4
                                js = slice(j * 512, (j + 1) * 512)
                                for i in range(2):
                                    dc = half * 2 + i
                                    nc.tensor.matmul(
                                        zt_ps[j][:],
                                        w_sb[:, dc * 128:(dc + 1) * 128],
                                        xt[i][:, js],
                                        start=(dc == 0), stop=(dc == 3))
                                if half == 1:
                                    nc.scalar.activation(zrR[:, js],
                                                         zt_ps[j][:],
                                                         Act.Copy)
                                    nc.scalar.activation(zrL[:, js],
                                                         zt_ps[j][:],
                                                         Act.Copy,
                                                         scale=-2.0)
                    # squared norms
                    zT2 = D2[:, 0:N]
                    nc.scalar.activation(zT2, zrR[:], Act.Square)
                    for j in range(NJT):
                        js = slice(j * 512, (j + 1) * 512)
                        sq_ps = pps1.tile([1, 512], F32, tag="small", name="sqps")
                        nc.tensor.matmul(sq_ps[:], ones_col[:], zT2[:, js],
                                         start=True, stop=True)
                        nc.vector.tensor_copy(sqrow[0:1, js], sq_ps[:])
                    for c in range(NCH):
                        tp = pps1.tile([128, 1], F32, tag="small", name="sqcolp")
                        nc.tensor.transpose(tp[:],
                                            sqrow[0:1, c * 128:(c + 1) * 128],
                                            id1[:])
                        nc.scalar.copy(sqcol[:, c:c + 1], tp[:])
                    # zdots_n = z_n . sum_m z_m
                    nc.vector.tensor_reduce(s_vec[:], zrR[:], axis=X_AX,
                                            op=Alu.add)
                    s_vR = mid.tile([128, 1], F32R, tag="s_vR", name="s_vR")
                    nc.vector.tensor_copy(s_vR[:], s_vec[:])
                    zdrow = D2[0:1, 3 * N:4 * N]
                    for j in range(NJT):
                        js = slice(j * 512, (j + 1) * 512)
                        zd_ps = pps1.tile([1, 512], F32, tag="small",
                                          name="zdps")
                        nc.tensor.matmul(zd_ps[:], s_vR[:], zrR[:, js],
                                         start=True, stop=True)
                        nc.vector.tensor_copy(zdrow[:, js], zd_ps[:])
                    for c in range(NCH):
                        tp = pps1.tile([128, 1], F32, tag="small", name="zdcp")
                        nc.tensor.transpose(
                            tp[:], zdrow[:, c * 128:(c + 1) * 128], id1[:])
                        nc.scalar.copy(zdots[:, c:c + 1], tp[:])
                    nc.vector.tensor_reduce(sc1[:], sqrow[0:1, :], axis=X_AX,
                                            op=Alu.add)
                    s1_ps = pps1.tile([128, 1], F32, tag="small", name="s1ps")
                    nc.tensor.matmul(s1_ps[:], ones_row[:], sc1[:],
                                     start=True, stop=True)
                    nc.scalar.activation(s1b[:], s1_ps[:], Act.Copy,
                                         scale=1.0 / N)
                    # global var(sq): scv = mean(sq^2) - mean(sq)^2
                    # (scratch rows live in not-yet-built D2 space)
                    sqsq = D2[0:1, N:2 * N]
                    nc.scalar.activation(sqsq, sqrow[0:1, :], Act.Square,
                                         accum_out=scv[:])
                    nc.vector.tensor_scalar(scv[:], scv[:], 1.0 / N,
                                            scalar2=None, op0=Alu.mult)
                    nc.vector.tensor_mul(sc3[:], sc1[:], sc1[:])
                    nc.vector.tensor_scalar(sc3[:], sc3[:], 1.0 / (N * N),
                                            scalar2=None, op0=Alu.mult)
                    nc.vector.tensor_sub(scv[:], scv[:], sc3[:])
                    vb_ps = pps1.tile([128, 1], F32, tag="small", name="vbps")
                    nc.tensor.matmul(vb_ps[:], ones_row[:], scv[:],
                                     start=True, stop=True)
                    nc.vector.tensor_copy(vglob[:], vb_ps[:])
                    # mu_n = sq_n + mean(sq) - (2/N) zdots_n
                    nc.vector.scalar_tensor_tensor(
                        out=mu[:], in0=zdots[:], scalar=-2.0 / N, in1=sqcol[:],
                        op0=Alu.mult, op1=Alu.add)
                    nc.vector.tensor_scalar(mu[:], mu[:], s1b[:], scalar2=None,
                                            op0=Alu.add)
                    # sig_n = sqrt(var(sq) + 4*sq_n*(mean_sq/128))
                    nc.vector.tensor_scalar(tmp3[:], sqcol[:], s1b[:],
                                            scalar2=4.0 / 128.0,
                                            op0=Alu.mult, op1=Alu.mult)
                    nc.vector.tensor_scalar(tmp3[:], tmp3[:], vglob[:],
                                            scalar2=None, op0=Alu.add)
                    nc.scalar.activation(sig[:], tmp3[:], Act.Sqrt)
                    # t0 = mu + ZEFF * sig ; slope = sig / (PHI * N)
                    nc.vector.scalar_tensor_tensor(
                        out=tcur[:], in0=sig[:], scalar=ZEFF, in1=mu[:],
                        op0=Alu.mult, op1=Alu.add)
                    nc.vector.tensor_scalar(slp[:], sig[:], 1.0 / (PHI * N),
                                            scalar2=None, op0=Alu.mult)
                    nc.vector.tensor_scalar(tneg[:], tcur[:], -1.0,
                                            scalar2=None, op0=Alu.mult)
                    # SQROWB = sq_m broadcast over partitions
                    for j in range(NJT):
                        js = slice(j * 512, (j + 1) * 512)
                        sb_ps = pps.tile([128, 512], F32, tag="big", name="sbps")
                        nc.tensor.matmul(sb_ps[:], ones_row[:],
                                         sqrow[0:1, js],
                                         start=True, stop=True)
                        nc.vector.tensor_copy(SQROWB[:, js], sb_ps[:])
                    # tracked hi init: last-resort fallback
                    nc.vector.memset(hiv[:], BIGT)
                    nc.vector.memset(chiv[:], float(N))

                # ---------------- build ----------------
                with tc.tile_pool(name="bscrv", bufs=2) as bscrv, \
                     tc.tile_pool(name="bscra", bufs=2) as bscra, \
                     tc.tile_pool(name="sqs", bufs=2) as sqsp, \
                     tc.tile_pool(name="bps", bufs=2, space="PSUM") as bps:
                    tc.strict_bb_all_engine_barrier()
                    for c in range(NCH):
                        d2c = D2[:, c * N:(c + 1) * N]
                        cs = slice(c * 128, (c + 1) * 128)
                        zz_ps = bps.tile([128, N], F32, tag="zz", name="zz")
                        for j in range(NJT):
                            js = slice(j * 512, (j + 1) * 512)
                            nc.tensor.matmul(zz_ps[:, js], zrL[:, cs],
                                             zrR[:, js], start=True,
                                             stop=True)
                        # psum = -2 z.z ; DVE adds sq_n + sq_m (no relu)
                        nc.vector.scalar_tensor_tensor(
                            out=d2c, in0=zz_ps[:], scalar=sqcol[:, c:c + 1],
                            in1=SQROWB[:], op0=Alu.add, op1=Alu.add)
                        sqs = sqsp.tile([128, 512], F32, tag="sqs", name="sqs")
                        nc.scalar.activation(
                            sqs[:, 0:256],
                            d2c.rearrange("p (a b) -> p a b", b=8)[:, :, 0:1],
                            Act.Sqrt, bias=c005[:],
                            accum_out=musum[:, c:c + 1])
                        # fused r0 count at t0 (per-engine scratch pools so
                        # ACT ops never queue on DVE-held buffers)
                        if CNT_ACT_BUILD(c):
                            cscr = bscra.tile([128, N], F32, tag="cscra",
                                              name="cscra")
                            nc.scalar.activation(cscr[:], d2c, Act.Sign,
                                                 bias=tneg[:, c:c + 1],
                                                 accum_out=sacc[:, c:c + 1])
                        else:
                            cscr = bscrv.tile([128, N], F32, tag="cscrv",
                                              name="cscrv")
                            nc.vector.tensor_scalar(
                                cscr[:], d2c, tcur[:, c:c + 1], scalar2=None,
                                op0=Alu.is_le, op1=Alu.add,
                                accum_out=ccur[:, c:c + 1])
                    if any(CNT_ACT_BUILD(c) for c in range(NCH)):
                        lo = min(c for c in range(NCH) if CNT_ACT_BUILD(c))
                        nc.vector.tensor_scalar(
                            ccur[:, lo:NCH], sacc[:, lo:NCH], -0.5,
                            scalar2=float(N) * 0.5, op0=Alu.mult, op1=Alu.add)

                # ---------------- mean distance ----------------
                with tc.tile_pool(name="mps", bufs=2, space="PSUM") as mps:
                    nc.vector.tensor_reduce(s_vec[:], musum[:], axis=X_AX,
                                            op=Alu.add)
                    ms_ps = mps.tile([1, 1], F32, tag="m", name="msps")
                    nc.tensor.matmul(ms_ps[:], ones_col[:], s_vec[:],
                                     start=True, stop=True)
                    nc.scalar.activation(sc2[:], ms_ps[:], Act.Copy,
                                         scale=1.0 / (N * 256.0))
                    nc.vector.tensor_reduce(
                        sc3[:],
                        sqrow[0:1, :].rearrange("p (a b) -> p a b", b=8)[:, :, 0:1],
                        axis=XY_AX, op=Alu.add)
                    nc.vector.tensor_scalar(sc3[:], sc3[:], 1.0 / 256.0,
                                            scalar2=None, op0=Alu.mult)
                    nc.vector.scalar_tensor_tensor(
                        out=sc4[:], in0=sc1[:], scalar=1.0 / N, in1=sc3[:],
                        op0=Alu.mult, op1=Alu.subtract)
                    nc.vector.tensor_scalar(sc3[:], sc2[:], 2.0, scalar2=None,
                                            op0=Alu.mult)
                    nc.vector.reciprocal(sc3[:], sc3[:])
                    nc.vector.tensor_mul(sc4[:], sc4[:], sc3[:])
                    nc.vector.tensor_add(sc2[:], sc2[:], sc4[:])
                    nc.vector.tensor_mul(sc2[:], sc2[:], sc2[:])
                    nc.vector.tensor_scalar(sc2[:], sc2[:], 2.0, scalar2=1e-8,
                                            op0=Alu.mult, op1=Alu.add)
                    nc.vector.reciprocal(sc2[:], sc2[:])
                    nc.vector.tensor_scalar(sc2[:], sc2[:], -1.0, scalar2=None,
                                            op0=Alu.mult)
                    ni_ps = mps.tile([128, 1], F32, tag="m", name="nips")
                    nc.tensor.matmul(ni_ps[:], ones_row[:], sc2[:],
                                     start=True, stop=True)
                    nc.vector.tensor_copy(neginvb[:], ni_ps[:])

            # ================= secant rounds =================
            simtp = tc.alloc_tile_pool(name="simtp", bufs=1)
            simt = [simtp.tile([128, N - c * 128], BF16, tag=f"simt{c}",
                               name=f"simt{c}") for c in range(NCH)]
            with tc.tile_pool(name="rscrv", bufs=2) as rscrv, \
                 tc.tile_pool(name="rscra", bufs=2) as rscra:
                tc.strict_bb_all_engine_barrier()
                for r in range(len(AIMS) + 1):
                    # merge (tcur, ccur) into tracked (hiv, chiv) where
                    # ccur >= K and tcur < hiv
                    nc.vector.tensor_scalar(mge[:], ccur[:], float(K),
                                            scalar2=None, op0=Alu.is_ge)
                    nc.vector.tensor_tensor(mbh[:], tcur[:], hiv[:],
                                            op=Alu.is_lt)
                    nc.vector.tensor_tensor(mnew[:], mge[:], mbh[:],
                                            op=Alu.logical_and)
                    nc.vector.select(hiv[:], mnew[:], tcur[:], hiv[:])
                    nc.vector.select(chiv[:], mnew[:], ccur[:], chiv[:])
                    if r == len(AIMS):
                        break
                    # t <- t + clamp((K + aim - c) * slp, +-STEP*sig)
                    nc.vector.tensor_scalar(tmp1[:], ccur[:], -1.0,
                                            scalar2=float(K) + AIMS[r],
                                            op0=Alu.mult, op1=Alu.add)
                    nc.vector.tensor_mul(tmp1[:], tmp1[:], slp[:])
                    nc.vector.tensor_scalar(tmp2[:], sig[:], STEP_CLAMP,
                                            scalar2=None, op0=Alu.mult)
                    nc.vector.tensor_tensor(tmp1[:], tmp1[:], tmp2[:],
                                            op=Alu.min)
                    nc.vector.tensor_scalar(tmp2[:], tmp2[:], -1.0,
                                            scalar2=None, op0=Alu.mult)
                    nc.vector.tensor_tensor(tmp1[:], tmp1[:], tmp2[:],
                                            op=Alu.max)
                    nc.vector.tensor_add(tcur[:], tcur[:], tmp1[:])
                    nc.vector.tensor_scalar(tneg[:], tcur[:], -1.0,
                                            scalar2=None, op0=Alu.mult)
                    # interleave DVE/ACT chunks so both engines start at once
                    ndve = sum(1 for c in range(NCH) if not CNT_ACT_ROUND(c))
                    for k in range(max(ndve, NCH - ndve)):
                        for c in ([k] if k < ndve else []) + \
                                 ([ndve + k] if ndve + k < NCH else []):
                            d2c = D2[:, c * N:(c + 1) * N]
                            if CNT_ACT_ROUND(c):
                                scr = rscra.tile([128, N], F32, tag="rscra",
                                                 name="rscra")
                                nc.scalar.activation(
                                    scr[:], d2c, Act.Sign,
                                    bias=tneg[:, c:c + 1],
                                    accum_out=sacc[:, c:c + 1])
                            else:
                                scr = rscrv.tile([128, N], F32, tag="rscrv",
                                                 name="rscrv")
                                nc.vector.tensor_scalar(
                                    scr[:], d2c, tcur[:, c:c + 1],
                                    scalar2=None,
                                    op0=Alu.is_le, op1=Alu.add,
                                    accum_out=ccur[:, c:c + 1])
                    lo = min(c for c in range(NCH) if CNT_ACT_ROUND(c))
                    nc.vector.tensor_scalar(
                        ccur[:, lo:NCH], sacc[:, lo:NCH], -0.5,
                        scalar2=float(N) * 0.5, op0=Alu.mult, op1=Alu.add)
                    if r < 4:
                        nc.scalar.activation(
                            simt[r][:],
                            D2[:, r * N + r * 128:r * N + N],
                            Act.Exp, bias=0.0, scale=neginvb[:])

            # ============ finisher + final (simt slabs persist) ============
            if True:
                # ---------------- finisher: top-8 below hi ----------------
                # s = sign(hi - d2) (+1 below); w = d2*s flips above-hi
                # values negative, so max8(w) is the top-8 below hi with
                # their exact d2 values. ACT also precomputes the exp slabs
                # (bf16) here, hiding them under the DVE/gpsimd work.
                with tc.tile_pool(name="sfin", bufs=4) as sfin, \
                     tc.tile_pool(name="wfin", bufs=3) as wfin:
                    for c in range(NCH):
                        d2c = D2[:, c * N:(c + 1) * N]
                        sscr = sfin.tile([128, N], F16, tag="sfin",
                                         name="sscr")
                        nc.scalar.activation(sscr[:], d2c, Act.Sign,
                                             bias=hiv[:, c:c + 1], scale=-1.0)
                        wscr = wfin.tile([128, N], F32, tag="wfin",
                                         name="wscr")
                        if FIN_GPS(c):
                            nc.gpsimd.tensor_mul(wscr[:], d2c, sscr[:])
                        else:
                            nc.vector.tensor_mul(wscr[:], d2c, sscr[:])
                        nc.vector.max(out=w8[:, c * 8:(c + 1) * 8],
                                      in_=wscr[:])
                        if c >= 4:
                            nc.scalar.activation(
                                simt[c][:],
                                D2[:, c * N + c * 128:c * N + N],
                                Act.Exp, bias=0.0, scale=neginvb[:])
                    # j = chi - K in [0,7] -> T = w8[j] ; else fallbacks
                    nc.vector.tensor_scalar(tmp1[:], chiv[:], 1.0,
                                            scalar2=-float(K),
                                            op0=Alu.mult, op1=Alu.add)
                    nc.vector.tensor_scalar(mge[:], tmp1[:], 0.0,
                                            scalar2=None, op0=Alu.is_ge)
                    nc.vector.tensor_scalar(mbh[:], tmp1[:], 7.0,
                                            scalar2=None, op0=Alu.is_le)
                    nc.vector.tensor_tensor(mok[:], mge[:], mbh[:],
                                            op=Alu.logical_and)
                    nc.vector.tensor_tensor(
                        ohsel[:].rearrange("p (c i) -> p c i", i=8),
                        iota8f[:].rearrange("p (c i) -> p c i", i=8),
                        tmp1[:].unsqueeze(2).to_broadcast([128, NCH, 8]),
                        op=Alu.is_equal)
                    nc.vector.tensor_mul(ohsel[:], ohsel[:], w8[:])
                    nc.vector.tensor_reduce(
                        tmp3[:], ohsel[:].rearrange("p (c i) -> p c i", i=8),
                        axis=X_AX, op=Alu.add)
                    # fallback: j>7 -> w8[7] (rank chi-7); j<0 -> hi
                    w87 = w8[:].rearrange("p (c i) -> p c i",
                                          i=8)[:, :, 7:8].squeeze(2)
                    nc.vector.select(tmp4[:], mge[:], w87, hiv[:])
                    nc.vector.select(Tfin[:], mok[:], tmp3[:], tmp4[:])

                # ================= final phase (upper slabs) =================
                with tc.tile_pool(name="fin1", bufs=1) as fin1, \
                     tc.tile_pool(name="msymp", bufs=4) as msymp, \
                     tc.tile_pool(name="fps", bufs=2, space="PSUM") as fps, \
                     tc.tile_pool(name="fps1", bufs=4, space="PSUM") as fps1:
                    TROWB = fin1.tile([128, N], F32, tag="TROWB", name="TROWB")
                    trow = fin1.tile([1, N], F32, tag="trow", name="trow")
                    for c in range(NCH):
                        tp = fps1.tile([1, 128], F32, tag="tfp", name="tfp")
                        nc.tensor.transpose(tp[:], Tfin[:, c:c + 1], ident[:])
                        nc.scalar.copy(trow[0:1, c * 128:(c + 1) * 128],
                                       tp[:])
                    for j in range(NJT):
                        js = slice(j * 512, (j + 1) * 512)
                        tb_ps = fps.tile([128, 512], F32, tag="tbps",
                                         name="tbps")
                        nc.tensor.matmul(tb_ps[:], ones_row[:],
                                         trow[0:1, js],
                                         start=True, stop=True)
                        nc.vector.tensor_copy(TROWB[:, js], tb_ps[:])
                    for c in range(NCH):
                        w = N - c * 128
                        lo = c * 128
                        d2s = D2[:, c * N + lo:c * N + N]
                        ms = msymp.tile([128, N], BF16, tag="ms", name="ms")
                        nc.vector.scalar_tensor_tensor(
                            out=ms[:, 0:w], in0=TROWB[:, lo:N],
                            scalar=Tfin[:, c:c + 1],
                            in1=d2s, op0=Alu.max, op1=Alu.is_ge)
                        if MUL_GPS(c):
                            nc.gpsimd.tensor_mul(ms[:, 0:w], ms[:, 0:w],
                                                 simt[c][:])
                        else:
                            nc.vector.tensor_mul(ms[:, 0:w], ms[:, 0:w],
                                                 simt[c][:])
                        if c < 8:
                            nc.sync.dma_start(outp[lo:lo + 64, lo:N],
                                              ms[0:64, 0:w])
                            nc.scalar.dma_start(outp[lo + 64:lo + 128, lo:N],
                                                ms[64:128, 0:w])
                        else:
                            nc.sync.dma_start(outp[lo:lo + 128, lo:N],
                                              ms[:, 0:w])
            simtp.release()
    nc.compile()
    return nc


_NC_CACHE = None
LAST_RESULTS = None


def _ensure_axon_hooks():
    """bass_utils imports antenv.axon_hooks when tracing; on images where
    the module is missing that import crashes the run. Provide a stub that
    reports no hook (tracing is skipped, results unaffected)."""
    try:
        import antenv.axon_hooks  # noqa: F401
    except Exception:
        import sys
        import types
        try:
            import antenv
        except Exception:
            return
        m = types.ModuleType("antenv.axon_hooks")
        m._HOOK = None
        m.set_axon_ntff_profile_hook = lambda h: setattr(m, "_HOOK", h)
        m.get_axon_ntff_profile_hook = lambda: m._HOOK
        sys.modules["antenv.axon_hooks"] = m


def _get_nc():
    global _NC_CACHE
    if _NC_CACHE is None:
        _NC_CACHE = build_nc()
    return _NC_CACHE


def kernel(x, W):
    _ensure_axon_hooks()
    from concourse.bass_utils import run_bass_kernel_spmd
    x = np.ascontiguousarray(np.asarray(x, dtype=np.float32))
    W = np.ascontiguousarray(np.asarray(W, dtype=np.float32))
    nc = _get_nc()
    in_maps = []
    for i in range(8):
        h, b = i // 2, i % 2
        in_maps.append({"xb": np.ascontiguousarray(x[b]),
                        "wh": np.ascontiguousarray(W[h])})
    res = run_bass_kernel_spmd(nc, in_maps, core_ids=list(range(8)))
    global LAST_RESULTS
    LAST_RESULTS = res
    adj = np.empty((2, N, N), dtype=np.float32)
    iu = np.triu_indices(N, 1)
    for b in range(2):
        acc = res.results[0 + b]["outp"].astype(np.float32)
        for h in range(1, 4):
            acc += res.results[2 * h + b]["outp"].astype(np.float32)
        acc[(iu[1], iu[0])] = acc[iu]
        adj[b] = acc * 0.25
    return adj
